# revision 10
# baseline (speedup 1.0000x reference)
"""Trainium2 Bass kernel for a CPC-style loss (graph pooling + NCE + distance).

Strategy (8 NeuronCores, SPMD):
  * Data-parallel pooling over seq_len: h_pool is only consumed through
    h_pool[start:end] (353 live rows), so only those rows are streamed
    (45/core); all 512 z rows are live (64/core).
  * Timestep-in-partition streaming layout: the host reshapes (no copy) each
    core's h shard to [90, 65536] (partition q = 2*s + node_half) and its z
    shard to [128, 32768], so every chunk DMA is a plain 2D slice with
    16-32KB fully contiguous runs per partition.  The mean over 1024 nodes
    becomes unit-stride halving-tree adds inside each partition (h tree on
    DVE, z tree on GPSIMD, both hidden under the DMA window), finished by a
    single 128-partition "pick" matmul per tensor that sums the two node
    halves of each timestep and applies 1/N:
        zmT[zi, s] = sum_q acc_z[q, zi] * pick[q, s]   (pick = 1/N at q//2==s)
        hmT[h, s]  = sum_q acc_h[q, h]  * pick[q, s]
  * z is streamed first (sync ring) and its pooled [64, 64] tile AllGathered
    early; the replicated z epilogue (projection, distance term,
    row-normalisation) is computed feature-major ([H, S]) so row norms are
    one ones-matmul column reduction - no transposes, no DRAM bounce.
  * The NCE is sharded over t_sample: each core scores only its own 45
    pooled-h timesteps (no h AllGather at all).  The per-core window of
    normalised z_pool columns is carved out of the replicated zhat with ONE
    dynamically-offset copy (offset register loaded from a per-core uint32
    input), after which all 14 shift slices are static.  Core 7's range is
    clipped to [END-45, END) and the 7 rows it shares with core 6 are zeroed
    via a per-core mask row baked into its consts input.
  * cosine sims feature-major: per shift one DVE multiply [H, 45] plus one
    ones-matmul column reduction into PSUM rows; log-softmax reduces to
    overlapping-window reductions on a [1, 630] row.
  * No final collective: each core returns raw partials (nce_sum, dist_sum)
    and kernel() sums/scales them on the host while unsharding.

The kernel function takes FULL unsharded inputs and returns the full output
tuple (nce_loss, distance), both float32 scalars.
"""

import os
import sys

import numpy as np

for _p in ("/opt/trn_rl_repo",):
    if _p not in sys.path and os.path.isdir(_p):
        sys.path.insert(0, _p)

import concourse.bacc as bacc
import concourse.bass as bass
import concourse.mybir as mybir
import concourse.tile as tile

F32 = mybir.dt.float32
U32 = mybir.dt.uint32
AX = mybir.AxisListType
OP = mybir.AluOpType
AF = mybir.ActivationFunctionType

# Problem constants (hardcoded; see module docstring).
S, N, H, Z = 512, 1024, 128, 64
NCORES = 8
SAMPLE_NUM, TIMESPAN = 8, 4
EPS = 1e-8
NEG_DIST = S // 6          # 85
END = S - SAMPLE_NUM - NEG_DIST - TIMESPAN + 2    # 417
START = S // 8             # 64
CNT = END - START          # 353
SZ = S // NCORES           # 64 z timesteps per core
SH = 45                    # h timesteps per core (t-shard width)
# shifts c = i + offs[m]; m=0 -> c=i (positives), m>=1 -> c=84+i+m in 86..95
SHIFTS = [1, 2, 3, 4] + list(range(86, 96))
NC14 = len(SHIFTS)         # 14
NPOS = TIMESPAN            # 4 positive shift blocks
NNEG = NC14 - NPOS         # 10 negative shift blocks
WWIN = SH + SHIFTS[-1] - 1  # 139: zhat cols [t0+1, t0+WWIN] cover all windows
TMAX = END - SH            # 372: largest per-core t0 (core 7, clipped)

# z streaming layout: partition q = 2*s + node_half (512 nodes per half)
ZCOLS = 512 * Z            # 32768 cols per z partition (512 nodes x 64 zi)
ZCHUNK = 64 * Z            # 4096-col z chunks (64 nodes)
NZCH = ZCOLS // ZCHUNK     # 8
# h streaming layout: partition p = node block (8 nodes), cols (s, u, h);
# full 128-partition DMA width, 4KB-contiguous runs per (partition, s)
NB = N // 128              # 8 nodes per partition
HS = 5                     # h timesteps per chunk
NHCH = SH // HS            # 9 chunks
HCHUNK = HS * NB * H       # 5120 cols per chunk

# packed-constants column layout (one [128, CW] input)
_C_WZT = 0          # [0:64, 0:128]   Wz.T
_C_WH = 128         # [:, 128:256]    Wh
_C_WPHIT = 256      # [:, 256:384]    Wphi.T
_C_BZ = 384         # [:, 384]        bz
_C_BH = 385         # [:, 385]        bh
_C_ONES = 386       # [:, 386]        ones column
_C_WVEC = 387       # [:, 387]        1/N column
_C_PICK = 388       # [:, 388:452]    pick[q, s] = 1/N where q//2 == s
_C_ONES_R = 452     # [0, 452:580]    ones row (128)
_C_BPHI = 580       # [0, 580:708]    bphi row
_C_MASK = 708       # [0, 708:753]    per-core t-validity mask row (45)
CW = 753


# timing-experiment switches (production = all False); set via profile scripts
_OPTS = {
    "streams_only": False,   # stop after pooling (wrong output; DMA phase only)
    "dma_only": False,       # with streams_only: skip pooling compute too
    "no_zag": False,         # skip z AllGather (wrong output; timing only)
}


def _emit(nc, tc, aps, reps=1):
    for _ in range(reps):
        _emit_once(nc, tc, aps)


def _emit_tree(eng, t, width, seg, tail, dst_final=None):
    """Halving-tree sum of [*, width] down to [*, seg] with unit-stride adds.

    ``t`` has ``width + width//2`` columns; folds ping-pong between column 0
    and column ``tail`` (= width) so no add overlaps its inputs.  The final
    fold writes ``dst_final`` if given (e.g. straight into the accumulator),
    else the result lands at t[:, src:src+seg] and (src, seg) is returned.
    """
    src, cur = 0, width
    while cur > seg:
        half = cur // 2
        if half == seg and dst_final is not None:
            eng.tensor_add(dst_final, t[:, src:src + half],
                           t[:, src + half:src + cur])
            return None
        dst = tail if src == 0 else 0
        eng.tensor_add(t[:, dst:dst + half], t[:, src:src + half],
                       t[:, src + half:src + cur])
        src, cur = dst, half
    return src


def _emit_once(nc, tc, aps):
    ahd, azd = aps["ahd"], aps["azd"]
    out = aps["out"]
    ag_groups = [list(range(NCORES))]

    with tc.tile_pool(name="const", bufs=1) as cpool, \
         tc.tile_pool(name="zstream", bufs=2) as zpool, \
         tc.tile_pool(name="hstream", bufs=2) as hpool, \
         tc.tile_pool(name="acc", bufs=1) as apool, \
         tc.tile_pool(name="work", bufs=2) as wpool, \
         tc.tile_pool(name="prod", bufs=4) as prodpool, \
         tc.tile_pool(name="dram", bufs=1, space="DRAM") as dpool, \
         tc.tile_pool(name="psumK", bufs=1, space="PSUM") as ppoolK:

        consts = cpool.tile([128, CW], F32, tag="consts")
        nc.sync.dma_start(consts[:], aps["consts"])
        wzt_sb = consts[0:Z, _C_WZT:_C_WZT + H]
        wh_sb = consts[:, _C_WH:_C_WH + H]
        wphit_sb = consts[:, _C_WPHIT:_C_WPHIT + H]
        bz_sb = consts[:, _C_BZ:_C_BZ + 1]
        bh_sb = consts[:, _C_BH:_C_BH + 1]
        ones_sb = consts[:, _C_ONES:_C_ONES + 1]
        wvec_sb = consts[:, _C_WVEC:_C_WVEC + 1]
        pick_sb = consts[:, _C_PICK:_C_PICK + SZ]
        ones_r_sb = consts[0:1, _C_ONES_R:_C_ONES_R + H]
        bphi_sb = consts[0:1, _C_BPHI:_C_BPHI + H]
        mask_sb = consts[0:1, _C_MASK:_C_MASK + SH]

        # h chunk DMA triggers first: the scalar ring carries nothing else
        # ahead of them, so h streams from t=0 concurrently with z (sync
        # ring).  Folds are emitted later (vector engine owns them).
        hbufs = []
        for g in range(NHCH):
            hbuf = hpool.tile([128, HCHUNK + HCHUNK // 2], F32, tag="hbuf")
            nc.scalar.dma_start(
                hbuf[:, 0:HCHUNK].rearrange("p (b f) -> p b f", b=HS),
                ahd[g * HS:(g + 1) * HS].rearrange("b p f -> p b f"))
            hbufs.append(hbuf)

        # distance accumulator (written once by the dssq matmul)
        psum_dist = ppoolK.tile([1, 1], F32, tag="psum_dist")

        # fused context weights WcT = (Wphi @ Wh).T and bias bc = Wphi@bh+bphi
        with tc.tile_pool(name="psumW", bufs=1, space="PSUM") as ppoolW:
            psum_wct = ppoolW.tile([H, H], F32, tag="psum_wct")
            nc.tensor.matmul(psum_wct[:], wh_sb, wphit_sb,
                             start=True, stop=True, skip_group_check=True)
            wct_sb = wpool.tile([H, H], F32, tag="wct_sb")
            nc.scalar.copy(wct_sb[:], psum_wct[:])
            psum_bc = ppoolW.tile([1, H], F32, tag="psum_bc")
            nc.tensor.matmul(psum_bc[:], bh_sb, wphit_sb,
                             start=True, stop=False, skip_group_check=True)
            nc.tensor.matmul(psum_bc[:], ones_sb[0:1, 0:1], bphi_sb,
                             start=False, stop=True, skip_group_check=True)
            bc_sb = wpool.tile([1, H], F32, tag="bc_sb")
            nc.scalar.copy(bc_sb[:], psum_bc[:])

        # ------------- z streaming (sync ring) + GPSIMD tree pooling -------
        acc_z = apool.tile([128, Z], F32, tag="acc_z")
        for g in range(NZCH):
            zbuf = zpool.tile([128, ZCHUNK + ZCHUNK // 2], F32, tag="zbuf")
            nc.sync.dma_start(zbuf[:, 0:ZCHUNK],
                              azd[:, g * ZCHUNK:(g + 1) * ZCHUNK])
            if _OPTS["dma_only"]:
                continue
            if g == 0:
                _emit_tree(nc.gpsimd, zbuf, ZCHUNK, Z, ZCHUNK,
                           dst_final=acc_z[:])
            else:
                src = _emit_tree(nc.gpsimd, zbuf, ZCHUNK, Z, ZCHUNK)
                nc.gpsimd.tensor_add(acc_z[:], acc_z[:],
                                     zbuf[:, src:src + Z])

        # ------------- early AllGather of the raw z accumulator ------------
        # acc_z is gathered unpooled ([128, 64] per core, 32KB) straight off
        # the GPSIMD tree via the sync ring, so no PE/scalar op delays it;
        # the pair-sum + transpose + 1/N happen post-gather via pick matmuls.
        accall = wpool.tile([128, NCORES * Z], F32, tag="accall")
        if _OPTS["no_zag"]:
            if not _OPTS["dma_only"]:
                for c in range(NCORES):
                    nc.gpsimd.tensor_copy(
                        out=accall[:, c * Z:(c + 1) * Z], in_=acc_z[:])
        else:
            cc_in_z = dpool.tile([1, 128 * Z], F32, tag="cc_in_z")
            nc.sync.dma_start(
                cc_in_z[0, :].rearrange("(q z) -> q z", q=128),
                acc_z[:] if not _OPTS["dma_only"]
                else consts[:, 0:Z])
            cc_out_z = dpool.tile([NCORES, 128 * Z], F32, tag="cc_out_z")
            nc.gpsimd.collective_compute(
                "AllGather", OP.bypass, replica_groups=ag_groups,
                ins=[cc_in_z[:].opt()], outs=[cc_out_z[:].opt()])
            nc.sync.dma_start(
                accall[:].rearrange("q (c z) -> q c z", c=NCORES),
                cc_out_z[:, :].rearrange("c (q z) -> q c z", q=128))

        # assemble replicated zmT [Z, S]: per core-block one pick matmul
        # (sums the two node halves of each timestep, transposes, scales 1/N)
        zmT = wpool.tile([Z, S], F32, tag="zmT")            # [z, s_global]
        with tc.tile_pool(name="psumZ", bufs=1, space="PSUM") as ppoolZ:
            if not _OPTS["dma_only"]:
                psum_zmT = ppoolZ.tile([Z, S], F32, tag="psum_zmT")
                for c in range(NCORES):
                    nc.tensor.matmul(
                        psum_zmT[:, c * SZ:(c + 1) * SZ],
                        accall[:, c * Z:(c + 1) * Z], pick_sb,
                        start=True, stop=True, skip_group_check=True)
                nc.scalar.copy(zmT[:], psum_zmT[:])

        if _OPTS["streams_only"]:
            _emit_h_pool(nc, tc, aps, hbufs, wvec_sb, None)
            out_sb0 = wpool.tile([1, 2], F32, tag="out_sb0")
            nc.scalar.copy(out_sb0[:], consts[0:1, 0:2])
            nc.sync.dma_start(out[:], out_sb0[:])
            return

        # ------------- z epilogue (replicated; overlaps h streaming) -------
        # Scalar+PE only on the zwin critical chain (the vector queue is
        # busy with h folds until the h stream drains).  Normalisation by
        # 1/||z_pool[s]|| is applied LATE, to the [1, 45]-block dot rows,
        # so no [H, S] zhat is ever materialised.  The torch eps clamp is
        # inactive for this input regime (row norms are O(10)), so rsqrt
        # replaces sqrt+max+reciprocal.
        zwin = wpool.tile([H, WWIN], F32, tag="zwin")
        nrzwin = wpool.tile([1, WWIN], F32, tag="nrzwin")
        with tc.tile_pool(name="psumE", bufs=1, space="PSUM") as ppoolE:
            psum_zp = ppoolE.tile([H, S], F32, tag="psum_zp")
            nc.tensor.matmul(psum_zp[:], wzt_sb, zmT[:],
                             start=True, stop=True, skip_group_check=True)
            zps = wpool.tile([H, S], F32, tag="zps")
            gsum = wpool.tile([H, 1], F32, tag="gsum")
            nc.scalar.activation(zps[:], psum_zp[:], AF.Identity,
                                 bias=bz_sb, accum_out=gsum[:])

            # distance = sum((z_pool - gmean)^2)  (host divides by S)
            gmean_n = wpool.tile([H, 1], F32, tag="gmean_n")
            nc.scalar.mul(gmean_n[:], gsum[:], -1.0 / S)
            zc = wpool.tile([H, S], F32, tag="zc")
            nc.scalar.activation(zc[:], zps[:], AF.Identity, bias=gmean_n)
            dsq = wpool.tile([H, S], F32, tag="dsq")
            dssq = wpool.tile([H, 1], F32, tag="dssq")
            nc.scalar.activation(dsq[:], zc[:], AF.Square, accum_out=dssq[:])
            nc.tensor.matmul(psum_dist[:], dssq[:], ones_sb,
                             start=True, stop=True, skip_group_check=True)

            # per-column 1/||z_pool[s]||, kept as a row for late application
            sqz = wpool.tile([H, S], F32, tag="sqz")
            nc.scalar.activation(sqz[:], zps[:], AF.Square)
            psum_zn = ppoolE.tile([1, S], F32, tag="psum_zn")
            nc.tensor.matmul(psum_zn[:], ones_sb, sqz[:],
                             start=True, stop=True, skip_group_check=True)
            nrz = wpool.tile([1, S], F32, tag="nrz")
            nc.scalar.sqrt(nrz[:], psum_zn[:])

            # per-core window [t0+1, t0+WWIN] of raw zps columns and of the
            # rz row; t0 comes from the per-core uint32 input, so one
            # dynamic slice covers all 14 static shift slices below.
            treg = nc.scalar.alloc_register("t0_reg%d" % nc.next_id())
            nc.scalar.reg_load(treg, aps["toff"][0:1, 0:1])
            tval = nc.scalar.snap(treg, donate=True, min_val=0, max_val=TMAX)
            nc.scalar.copy(zwin[:], zps[:, 1:S][:, bass.ds(tval, WWIN)])
            nc.scalar.copy(nrzwin[:], nrz[0:1, 1:S][:, bass.ds(tval, WWIN)])

        # ------------- h pooling (vector folds + PE matvecs) ---------------
        hmT_sb = _emit_h_pool(nc, tc, aps, hbufs, wvec_sb, wpool)

        # ------------- NCE (t-sharded; local h only) -----------------------
        with tc.tile_pool(name="psumC", bufs=1, space="PSUM") as ppoolC:
            # context projection, feature-major [H, SH]
            psum_cph = ppoolC.tile([H, SH], F32, tag="psum_cph")
            nc.tensor.matmul(psum_cph[:], wct_sb[:], hmT_sb[:],
                             start=True, stop=False, skip_group_check=True)
            nc.tensor.matmul(psum_cph[:], bc_sb[:], ones_r_sb[0:1, 0:SH],
                             start=False, stop=True, skip_group_check=True)

            # column norms -> cphihat = c_phi * 1/||c_phi[t]||
            sqc = wpool.tile([H, SH], F32, tag="sqc")
            nc.scalar.activation(sqc[:], psum_cph[:], AF.Square)
            psum_cn = ppoolC.tile([1, SH], F32, tag="psum_cn")
            nc.tensor.matmul(psum_cn[:], ones_sb, sqc[:],
                             start=True, stop=True, skip_group_check=True)
            nrc = wpool.tile([1, SH], F32, tag="nrc")
            nc.scalar.sqrt(nrc[:], psum_cn[:])
            rc = wpool.tile([1, SH], F32, tag="rc")
            nc.vector.reciprocal(rc[:], nrc[:])
            psum_rcb = ppoolC.tile([H, SH], F32, tag="psum_rcb")
            nc.tensor.matmul(psum_rcb[:], ones_r_sb, rc[:],
                             start=True, stop=True, skip_group_check=True)
            rcb = wpool.tile([H, SH], F32, tag="rcb")
            nc.scalar.copy(rcb[:], psum_rcb[:])
            cph = wpool.tile([H, SH], F32, tag="cph")
            nc.vector.tensor_tensor(out=cph[:], in0=psum_cph[:], in1=rcb[:],
                                    op=OP.mult)

            # cosine sims: per shift one DVE multiply + one ones-matmul
            # column reduction; results land as [1, 45] blocks in PSUM,
            # then the rz window rows normalise them block-strided.
            psum_dp = ppoolC.tile([1, NPOS * SH], F32, tag="psum_dp")
            psum_dn = ppoolC.tile([1, NNEG * SH], F32, tag="psum_dn")
            for j, delta in enumerate(SHIFTS):
                prod = prodpool.tile([H, SH], F32, tag="prod")
                nc.vector.tensor_tensor(
                    out=prod[:], in0=zwin[:, delta - 1:delta - 1 + SH],
                    in1=cph[:], op=OP.mult)
                dst = (psum_dp[:, j * SH:(j + 1) * SH] if j < NPOS
                       else psum_dn[:, (j - NPOS) * SH:(j - NPOS + 1) * SH])
                nc.tensor.matmul(dst, ones_sb, prod[:],
                                 start=True, stop=True, skip_group_check=True)
            rzwin = wpool.tile([1, WWIN], F32, tag="rzwin")
            nc.vector.reciprocal(rzwin[:], nrzwin[:])
            rw = rzwin[:]
            dpos = wpool.tile([1, NPOS * SH], F32, tag="dpos")
            nc.vector.tensor_tensor(
                out=dpos[:], in0=psum_dp[:],
                in1=bass.AP(rw.tensor, rw.offset,
                            [[rw.ap[0][0], 1], [1, NPOS], [1, SH]]),
                op=OP.mult)
            dneg = wpool.tile([1, NNEG * SH], F32, tag="dneg")
            nc.vector.tensor_tensor(
                out=dneg[:], in0=psum_dn[:],
                in1=bass.AP(rw.tensor, rw.offset + SHIFTS[NPOS] - 1,
                            [[rw.ap[0][0], 1], [1, NNEG], [1, SH]]),
                op=OP.mult)

            # log-softmax over the 8 samples; positive at m=0
            expd = wpool.tile([1, NC14 * SH], F32, tag="expd")
            nc.scalar.activation(expd[0:1, 0:NPOS * SH], dpos[:], AF.Exp)
            nc.scalar.activation(expd[0:1, NPOS * SH:NC14 * SH], dneg[:],
                                 AF.Exp)
            den = wpool.tile([1, NPOS * SH], F32, tag="den")
            eb = expd[:]
            for ii in range(TIMESPAN):
                # negatives for step ii live in 7 consecutive 45-blocks
                neg_ap = bass.AP(
                    eb.tensor, eb.offset + (NPOS + ii) * SH,
                    [[eb.ap[0][0], 1], [1, SH], [SH, 7]])
                nc.vector.reduce_sum(den[0:1, ii * SH:(ii + 1) * SH],
                                     neg_ap, axis=AX.X)
            nc.vector.tensor_add(den[:], den[:], expd[0:1, 0:NPOS * SH])
            lse = wpool.tile([1, NPOS * SH], F32, tag="lse")
            nc.scalar.activation(lse[:], den[:], AF.Ln)
            ctr = wpool.tile([1, NPOS * SH], F32, tag="ctr")
            nc.vector.tensor_sub(ctr[:], dpos[:], lse[:])
            cb = ctr[:]
            ctrt = wpool.tile([1, SH], F32, tag="ctrt")
            sum_ap = bass.AP(cb.tensor, cb.offset,
                             [[cb.ap[0][0], 1], [1, SH], [SH, TIMESPAN]])
            nc.vector.reduce_sum(ctrt[:], sum_ap, axis=AX.X)
            masked = wpool.tile([1, SH], F32, tag="masked")
            nc.vector.tensor_tensor(out=masked[:], in0=ctrt[:], in1=mask_sb,
                                    op=OP.mult)
            nce1 = wpool.tile([1, 1], F32, tag="nce1")
            nc.vector.reduce_sum(nce1[:], masked[:], axis=AX.X)

        # ------------- raw partials out (host sums across cores) -----------
        out_sb = wpool.tile([1, 2], F32, tag="out_sb")
        nc.vector.tensor_copy(out=out_sb[0:1, 0:1], in_=nce1[:])
        nc.scalar.copy(out_sb[0:1, 1:2], psum_dist[:])
        nc.sync.dma_start(out[:], out_sb[:])


def _ap_of(t, off, dims):
    b = t[:]
    return bass.AP(b.tensor, b.offset + off, [[b.ap[0][0], b.ap[0][1]]] + dims)


def _emit_h_pool(nc, tc, aps, hbufs, wvec_sb, wpool):
    """Segmented unit-stride folds (8 nodes/partition -> 1) on the vector
    engine, then one [128]x[128,1] matvec per timestep on PE.  Returns
    hmT_sb [H, SH] (the node-mean, transposed)."""
    with tc.tile_pool(name="hacc", bufs=1) as hapool, \
         tc.tile_pool(name="psumH", bufs=1, space="PSUM") as ppoolH:
        hacc = hapool.tile([128, SH * H], F32, tag="hacc")
        psum_hmT = ppoolH.tile([H, SH], F32, tag="psum_hmT")
        for g, hbuf in enumerate(hbufs):
            if _OPTS["dma_only"]:
                continue
            # fold u: 8 -> 4 (into the tail region), 4 -> 2 (back to base),
            # 2 -> 1 (straight into hacc)
            nc.vector.tensor_add(
                _ap_of(hbuf, HCHUNK, [[4 * H, HS], [1, 4 * H]]),
                _ap_of(hbuf, 0, [[NB * H, HS], [1, 4 * H]]),
                _ap_of(hbuf, 4 * H, [[NB * H, HS], [1, 4 * H]]))
            nc.vector.tensor_add(
                _ap_of(hbuf, 0, [[2 * H, HS], [1, 2 * H]]),
                _ap_of(hbuf, HCHUNK, [[4 * H, HS], [1, 2 * H]]),
                _ap_of(hbuf, HCHUNK + 2 * H, [[4 * H, HS], [1, 2 * H]]))
            nc.vector.tensor_add(
                _ap_of(hacc, g * HS * H, [[H, HS], [1, H]]),
                _ap_of(hbuf, 0, [[2 * H, HS], [1, H]]),
                _ap_of(hbuf, H, [[2 * H, HS], [1, H]]))
            for k in range(HS):
                s = g * HS + k
                nc.tensor.matmul(
                    psum_hmT[:, s:s + 1],
                    hacc[:, s * H:(s + 1) * H], wvec_sb,
                    start=True, stop=True, skip_group_check=True)
        if _OPTS["dma_only"] or wpool is None:
            return None
        hmT_sb = wpool.tile([H, SH], F32, tag="hmT_sb")
        nc.scalar.copy(hmT_sb[:], psum_hmT[:])
        return hmT_sb


def _build(reps=1):
    nc = bacc.Bacc("TRN2", debug=False, enable_asserts=False,
                   target_bir_lowering=False, num_devices=NCORES)
    aps = {}

    def din(name, shape, dt=F32):
        aps[name] = nc.dram_tensor(name, shape, dt, kind="ExternalInput").ap()

    din("ahd", [SH, 128, NB * H])
    din("azd", [128, ZCOLS])
    din("consts", [128, CW])
    din("toff", [1, 1], U32)
    aps["out"] = nc.dram_tensor("out", [1, 2], F32,
                                kind="ExternalOutput").ap()

    with tile.TileContext(nc) as tc:
        _emit(nc, tc, aps, reps=reps)
    nc.compile()
    return nc


_CACHE = {}


def _core_t0(c):
    """Global first t_sample of core c's shard (core 7 clipped to END-45)."""
    return min(START + SH * c, END - SH)


def _pack_consts(Wh, bh, Wz, bz, Wphi, bphi, core):
    c = np.zeros((128, CW), dtype=np.float32)
    c[0:Z, _C_WZT:_C_WZT + H] = Wz.T
    c[:, _C_WH:_C_WH + H] = Wh
    c[:, _C_WPHIT:_C_WPHIT + H] = Wphi.T
    c[:, _C_BZ] = bz
    c[:, _C_BH] = bh
    c[:, _C_ONES] = 1.0
    c[:, _C_WVEC] = 1.0 / N
    for q in range(128):
        c[q, _C_PICK + q // 2] = 1.0 / N
    c[0, _C_ONES_R:_C_ONES_R + H] = 1.0
    c[0, _C_BPHI:_C_BPHI + H] = bphi
    # rows whose global index is owned by a lower core are masked off
    # (only core 7's clipped shard overlaps core 6's)
    t0 = _core_t0(core)
    m = (np.arange(t0, t0 + SH) >= START + SH * core).astype(np.float32)
    c[0, _C_MASK:_C_MASK + SH] = m
    return c


def make_in_maps(all_h, all_z, Wh, bh, Wz, bz, Wphi, bphi):
    in_maps = []
    for c in range(NCORES):
        t0 = _core_t0(c)
        in_maps.append({
            "consts": _pack_consts(Wh, bh, Wz, bz, Wphi, bphi, c),
            "toff": np.array([[t0]], dtype=np.uint32),
            "ahd": np.ascontiguousarray(
                all_h[t0:t0 + SH]).reshape(SH, 128, NB * H),
            "azd": np.ascontiguousarray(
                all_z[c * SZ:(c + 1) * SZ]).reshape(128, ZCOLS),
        })
    return in_maps


def _get_runner():
    """Build the Bass program and one jitted shard_map executable, once.

    Re-lowering a fresh executable per call reloads the collective NEFF and
    leaves NRT unrecoverable on the second call, so the executable is cached
    and every kernel() invocation reuses it with freshly uploaded inputs.
    """
    if "runner" in _CACHE:
        return _CACHE["runner"]

    import jax
    from concourse import bass2jax
    from concourse.bass2jax import _bass_exec_p, partition_id_tensor
    from jax.sharding import Mesh, PartitionSpec, NamedSharding
    from jax.experimental.shard_map import shard_map

    nc = _build()
    bass2jax.install_neuronx_cc_hook()
    partition_name = (nc.partition_id_tensor.name
                      if nc.partition_id_tensor else None)

    in_names, out_names, out_avals, zero_outs = [], [], [], []
    for alloc in nc.m.functions[0].allocations:
        if not isinstance(alloc, mybir.MemoryLocationSet):
            continue
        name = alloc.memorylocations[0].name
        if alloc.kind == "ExternalInput":
            if name != partition_name:
                in_names.append(name)
        elif alloc.kind == "ExternalOutput":
            shape = tuple(alloc.tensor_shape)
            dtype = mybir.dt.np(alloc.dtype)
            out_names.append(name)
            out_avals.append(jax.core.ShapedArray(shape, dtype))
            zero_outs.append(np.zeros(shape, dtype))
    n_params = len(in_names)
    all_in_names = list(in_names) + out_names
    if partition_name is not None:
        all_in_names.append(partition_name)

    def _body(*args):
        operands = list(args)
        if partition_name is not None:
            operands.append(partition_id_tensor())
        outs = _bass_exec_p.bind(
            *operands,
            out_avals=tuple(out_avals),
            in_names=tuple(all_in_names),
            out_names=tuple(out_names),
            lowering_input_output_aliases=(),
            sim_require_finite=True,
            sim_require_nnan=True,
            nc=nc,
        )
        return tuple(outs)

    devices = jax.devices()[:NCORES]
    mesh = Mesh(np.asarray(devices), ("core",))
    n_outs = len(out_avals)
    in_specs = (PartitionSpec("core"),) * (n_params + n_outs)
    out_specs = (PartitionSpec("core"),) * n_outs
    sharded = jax.jit(shard_map(_body, mesh=mesh, in_specs=in_specs,
                                out_specs=out_specs, check_rep=False),
                      keep_unused=True)
    sh = NamedSharding(mesh, PartitionSpec("core"))
    dev_zeros = [
        jax.device_put(
            np.zeros((NCORES * z.shape[0], *z.shape[1:]), z.dtype), sh)
        for z in zero_outs
    ]

    def run(in_maps):
        dev_in = [
            jax.device_put(
                np.concatenate([np.asarray(in_maps[c][n])
                                for c in range(NCORES)], axis=0), sh)
            for n in in_names
        ]
        outs = sharded(*dev_in, *dev_zeros)
        return [
            {name: np.asarray(outs[i]).reshape(NCORES, *out_avals[i].shape)[c]
             for i, name in enumerate(out_names)}
            for c in range(NCORES)
        ]

    _CACHE["runner"] = run
    return run


def kernel(all_h, all_z, Wh, bh, Wz, bz, Wphi, bphi):
    all_h = np.ascontiguousarray(np.asarray(all_h, dtype=np.float32))
    all_z = np.ascontiguousarray(np.asarray(all_z, dtype=np.float32))
    args = [np.asarray(x, dtype=np.float32)
            for x in (Wh, bh, Wz, bz, Wphi, bphi)]

    # The axon NTFF trace hook (antenv.axon_hooks) is absent in this image;
    # make sure an inherited BASS_TRACE can't route us onto that path.
    os.environ["BASS_NEVER_TRACE"] = "1"

    run = _get_runner()
    in_maps = make_in_maps(all_h, all_z, *args)
    results = run(in_maps)
    _CACHE["last_results"] = results

    # unshard: sum per-core NCE partials, average the replicated distance
    nce_sum = float(sum(results[c]["out"][0, 0] for c in range(NCORES)))
    dist_sum = float(sum(results[c]["out"][0, 1] for c in range(NCORES)))
    nce_loss = np.float32(nce_sum * (-1.0 / (CNT * TIMESPAN)))
    distance = np.float32(dist_sum / (NCORES * S))
    return (nce_loss, distance)


# revision 11
# speedup vs baseline: 1.0493x; 1.0493x over previous
"""Trainium2 Bass kernel for a CPC-style loss (graph pooling + NCE + distance).

Strategy (8 NeuronCores, SPMD), ~110us/body vs the ~97.6us HBM roofline
(40.4 MB/core at ~414 GB/s):
  * Data-parallel pooling over seq_len: h_pool is only consumed through
    h_pool[start:end] (353 live rows), so only those rows are streamed
    (45/core); all 512 z rows are live (64/core).
  * Both streams run concurrently from t=0 on separate DMA rings at full
    128-partition width (DMA bandwidth scales with partitions used):
    z on the sync ring as [128, 32768] (partition q = 2*s + node_half,
    16KB contiguous runs), h on the scalar ring as [45, 128, 1024]
    (partition = 8-node block, 4KB runs).  Engine queues are in-order, so
    the h DMA triggers are emitted FIRST on an otherwise-empty scalar
    ring and nothing ever stalls them.
  * Node-mean compute hides entirely under the DMA window as unit-stride
    halving-tree adds: z chunks tree 64n->1 on GPSIMD into acc_z [128, 64];
    h chunks fold 8n->1 on DVE (3 big segmented adds per 5-timestep chunk)
    into hacc, finished by one [128]x[128,1] PE matvec per timestep.
  * The raw acc_z is AllGathered ([128, 64] per core, 32KB) straight off
    the GPSIMD tree via sync-ring staging - no PE/scalar op delays the
    collective, which hides under the h stream.  Post-gather, one "pick"
    matmul per core-block (pick[q, s] = 1/N at q//2 == s) sums the node
    halves, transposes, and scales: zmT[zi, s] = sum_q acc[q, zi] pick[q, s].
  * The replicated z epilogue (projection, distance, norms) keeps
    scalar+PE-only ops on its critical chain (the vector queue is busy
    with h folds): row sums ride the activation's accum_out, norms are
    ones-matmul column reductions, and 1/||z_pool|| is applied LATE to the
    [1, 45] dot-product rows (block-strided), so no [H, S] zhat exists and
    no vector op gates the window slice.  The torch eps clamp is inactive
    for this input regime (row norms are O(10)) and is elided.
  * The NCE is sharded over t_sample: each core scores only its own 45
    pooled-h timesteps (no h AllGather at all).  The per-core window of
    raw z_pool columns (and of the norm row) is carved out with ONE
    dynamically-offset copy (offset register from a per-core uint32
    input), after which all 14 shift slices are static.  Core 7's range is
    clipped to [END-45, END) and the 7 rows it shares with core 6 are
    zeroed via a per-core mask row baked into its consts input.
  * cosine sims feature-major: per shift one DVE multiply [H, 45] plus one
    ones-matmul column reduction into PSUM rows; log-softmax reduces to
    overlapping-window reductions on a [1, 630] row.
  * No final collective: each core returns raw partials (nce_sum, dist_sum)
    and kernel() sums/scales them on the host while unsharding.

The kernel function takes FULL unsharded inputs and returns the full output
tuple (nce_loss, distance), both float32 scalars.
"""

import os
import sys

import numpy as np

for _p in ("/opt/trn_rl_repo",):
    if _p not in sys.path and os.path.isdir(_p):
        sys.path.insert(0, _p)

import concourse.bacc as bacc
import concourse.bass as bass
import concourse.mybir as mybir
import concourse.tile as tile

F32 = mybir.dt.float32
U32 = mybir.dt.uint32
AX = mybir.AxisListType
OP = mybir.AluOpType
AF = mybir.ActivationFunctionType

# Problem constants (hardcoded; see module docstring).
S, N, H, Z = 512, 1024, 128, 64
NCORES = 8
SAMPLE_NUM, TIMESPAN = 8, 4
EPS = 1e-8
NEG_DIST = S // 6          # 85
END = S - SAMPLE_NUM - NEG_DIST - TIMESPAN + 2    # 417
START = S // 8             # 64
CNT = END - START          # 353
SZ = S // NCORES           # 64 z timesteps per core
SH = 45                    # h timesteps per core (t-shard width)
# shifts c = i + offs[m]; m=0 -> c=i (positives), m>=1 -> c=84+i+m in 86..95
SHIFTS = [1, 2, 3, 4] + list(range(86, 96))
NC14 = len(SHIFTS)         # 14
NPOS = TIMESPAN            # 4 positive shift blocks
NNEG = NC14 - NPOS         # 10 negative shift blocks
WWIN = SH + SHIFTS[-1] - 1  # 139: zhat cols [t0+1, t0+WWIN] cover all windows
TMAX = END - SH            # 372: largest per-core t0 (core 7, clipped)

# z streaming layout: partition q = 2*s + node_half (512 nodes per half)
ZCOLS = 512 * Z            # 32768 cols per z partition (512 nodes x 64 zi)
ZCHUNK = 64 * Z            # 4096-col z chunks (64 nodes)
NZCH = ZCOLS // ZCHUNK     # 8
# h streaming layout: partition p = node block (8 nodes), cols (s, u, h);
# full 128-partition DMA width, 4KB-contiguous runs per (partition, s)
NB = N // 128              # 8 nodes per partition
HS = 5                     # h timesteps per chunk
NHCH = SH // HS            # 9 chunks
HCHUNK = HS * NB * H       # 5120 cols per chunk

# packed-constants column layout (one [128, CW] input)
_C_WZT = 0          # [0:64, 0:128]   Wz.T
_C_WH = 128         # [:, 128:256]    Wh
_C_WPHIT = 256      # [:, 256:384]    Wphi.T
_C_BZ = 384         # [:, 384]        bz
_C_BH = 385         # [:, 385]        bh
_C_ONES = 386       # [:, 386]        ones column
_C_WVEC = 387       # [:, 387]        1/N column
_C_PICK = 388       # [:, 388:452]    pick[q, s] = 1/N where q//2 == s
_C_ONES_R = 452     # [0, 452:580]    ones row (128)
_C_BPHI = 580       # [0, 580:708]    bphi row
_C_MASK = 708       # [0, 708:753]    per-core t-validity mask row (45)
CW = 753


# timing-experiment switches (production = all False); set via profile scripts
_OPTS = {
    "streams_only": False,   # stop after pooling (wrong output; DMA phase only)
    "dma_only": False,       # with streams_only: skip pooling compute too
    "no_zag": False,         # skip z AllGather (wrong output; timing only)
}


def _emit(nc, tc, aps, reps=1):
    for _ in range(reps):
        _emit_once(nc, tc, aps)


def _emit_tree(eng, t, width, seg, tail, dst_final=None):
    """Halving-tree sum of [*, width] down to [*, seg] with unit-stride adds.

    ``t`` has ``width + width//2`` columns; folds ping-pong between column 0
    and column ``tail`` (= width) so no add overlaps its inputs.  The final
    fold writes ``dst_final`` if given (e.g. straight into the accumulator),
    else the result lands at t[:, src:src+seg] and (src, seg) is returned.
    """
    src, cur = 0, width
    while cur > seg:
        half = cur // 2
        if half == seg and dst_final is not None:
            eng.tensor_add(dst_final, t[:, src:src + half],
                           t[:, src + half:src + cur])
            return None
        dst = tail if src == 0 else 0
        eng.tensor_add(t[:, dst:dst + half], t[:, src:src + half],
                       t[:, src + half:src + cur])
        src, cur = dst, half
    return src


def _emit_once(nc, tc, aps):
    ahd, azd = aps["ahd"], aps["azd"]
    out = aps["out"]
    ag_groups = [list(range(NCORES))]

    with tc.tile_pool(name="const", bufs=1) as cpool, \
         tc.tile_pool(name="zstream", bufs=2) as zpool, \
         tc.tile_pool(name="hstream", bufs=2) as hpool, \
         tc.tile_pool(name="acc", bufs=1) as apool, \
         tc.tile_pool(name="work", bufs=2) as wpool, \
         tc.tile_pool(name="prod", bufs=4) as prodpool, \
         tc.tile_pool(name="dram", bufs=1, space="DRAM") as dpool, \
         tc.tile_pool(name="psumK", bufs=1, space="PSUM") as ppoolK:

        consts = cpool.tile([128, CW], F32, tag="consts")
        nc.sync.dma_start(consts[:], aps["consts"])
        wzt_sb = consts[0:Z, _C_WZT:_C_WZT + H]
        wh_sb = consts[:, _C_WH:_C_WH + H]
        wphit_sb = consts[:, _C_WPHIT:_C_WPHIT + H]
        bz_sb = consts[:, _C_BZ:_C_BZ + 1]
        bh_sb = consts[:, _C_BH:_C_BH + 1]
        ones_sb = consts[:, _C_ONES:_C_ONES + 1]
        wvec_sb = consts[:, _C_WVEC:_C_WVEC + 1]
        pick_sb = consts[:, _C_PICK:_C_PICK + SZ]
        ones_r_sb = consts[0:1, _C_ONES_R:_C_ONES_R + H]
        bphi_sb = consts[0:1, _C_BPHI:_C_BPHI + H]
        mask_sb = consts[0:1, _C_MASK:_C_MASK + SH]

        # h chunk DMA triggers first: the scalar ring carries nothing else
        # ahead of them, so h streams from t=0 concurrently with z (sync
        # ring).  Folds are emitted later (vector engine owns them).
        hbufs = []
        for g in range(NHCH):
            hbuf = hpool.tile([128, HCHUNK + HCHUNK // 2], F32, tag="hbuf")
            nc.scalar.dma_start(
                hbuf[:, 0:HCHUNK].rearrange("p (b f) -> p b f", b=HS),
                ahd[g * HS:(g + 1) * HS].rearrange("b p f -> p b f"))
            hbufs.append(hbuf)

        # distance accumulator (written once by the dssq matmul)
        psum_dist = ppoolK.tile([1, 1], F32, tag="psum_dist")

        # fused context weights WcT = (Wphi @ Wh).T and bias bc = Wphi@bh+bphi
        with tc.tile_pool(name="psumW", bufs=1, space="PSUM") as ppoolW:
            psum_wct = ppoolW.tile([H, H], F32, tag="psum_wct")
            nc.tensor.matmul(psum_wct[:], wh_sb, wphit_sb,
                             start=True, stop=True, skip_group_check=True)
            wct_sb = wpool.tile([H, H], F32, tag="wct_sb")
            nc.scalar.copy(wct_sb[:], psum_wct[:])
            psum_bc = ppoolW.tile([1, H], F32, tag="psum_bc")
            nc.tensor.matmul(psum_bc[:], bh_sb, wphit_sb,
                             start=True, stop=False, skip_group_check=True)
            nc.tensor.matmul(psum_bc[:], ones_sb[0:1, 0:1], bphi_sb,
                             start=False, stop=True, skip_group_check=True)
            bc_sb = wpool.tile([1, H], F32, tag="bc_sb")
            nc.scalar.copy(bc_sb[:], psum_bc[:])

        # ------------- z streaming (sync ring) + GPSIMD tree pooling -------
        acc_z = apool.tile([128, Z], F32, tag="acc_z")
        for g in range(NZCH):
            zbuf = zpool.tile([128, ZCHUNK + ZCHUNK // 2], F32, tag="zbuf")
            nc.sync.dma_start(zbuf[:, 0:ZCHUNK],
                              azd[:, g * ZCHUNK:(g + 1) * ZCHUNK])
            if _OPTS["dma_only"]:
                continue
            if g == 0:
                _emit_tree(nc.gpsimd, zbuf, ZCHUNK, Z, ZCHUNK,
                           dst_final=acc_z[:])
            else:
                src = _emit_tree(nc.gpsimd, zbuf, ZCHUNK, Z, ZCHUNK)
                nc.gpsimd.tensor_add(acc_z[:], acc_z[:],
                                     zbuf[:, src:src + Z])

        # ------------- early AllGather of the raw z accumulator ------------
        # acc_z is gathered unpooled ([128, 64] per core, 32KB) straight off
        # the GPSIMD tree via the sync ring, so no PE/scalar op delays it;
        # the pair-sum + transpose + 1/N happen post-gather via pick matmuls.
        accall = wpool.tile([128, NCORES * Z], F32, tag="accall")
        if _OPTS["no_zag"]:
            if not _OPTS["dma_only"]:
                for c in range(NCORES):
                    nc.gpsimd.tensor_copy(
                        out=accall[:, c * Z:(c + 1) * Z], in_=acc_z[:])
        else:
            cc_in_z = dpool.tile([1, 128 * Z], F32, tag="cc_in_z")
            nc.sync.dma_start(
                cc_in_z[0, :].rearrange("(q z) -> q z", q=128),
                acc_z[:] if not _OPTS["dma_only"]
                else consts[:, 0:Z])
            cc_out_z = dpool.tile([NCORES, 128 * Z], F32, tag="cc_out_z")
            nc.gpsimd.collective_compute(
                "AllGather", OP.bypass, replica_groups=ag_groups,
                ins=[cc_in_z[:].opt()], outs=[cc_out_z[:].opt()])
            nc.sync.dma_start(
                accall[:].rearrange("q (c z) -> q c z", c=NCORES),
                cc_out_z[:, :].rearrange("c (q z) -> q c z", q=128))

        # assemble replicated zmT [Z, S]: per core-block one pick matmul
        # (sums the two node halves of each timestep, transposes, scales 1/N)
        zmT = wpool.tile([Z, S], F32, tag="zmT")            # [z, s_global]
        with tc.tile_pool(name="psumZ", bufs=1, space="PSUM") as ppoolZ:
            if not _OPTS["dma_only"]:
                psum_zmT = ppoolZ.tile([Z, S], F32, tag="psum_zmT")
                for c in range(NCORES):
                    nc.tensor.matmul(
                        psum_zmT[:, c * SZ:(c + 1) * SZ],
                        accall[:, c * Z:(c + 1) * Z], pick_sb,
                        start=True, stop=True, skip_group_check=True)
                nc.scalar.copy(zmT[:], psum_zmT[:])

        if _OPTS["streams_only"]:
            _emit_h_pool(nc, tc, aps, hbufs, wvec_sb, None)
            out_sb0 = wpool.tile([1, 2], F32, tag="out_sb0")
            nc.scalar.copy(out_sb0[:], consts[0:1, 0:2])
            nc.sync.dma_start(out[:], out_sb0[:])
            return

        # ------------- z epilogue (replicated; overlaps h streaming) -------
        # Scalar+PE only on the zwin critical chain (the vector queue is
        # busy with h folds until the h stream drains).  Normalisation by
        # 1/||z_pool[s]|| is applied LATE, to the [1, 45]-block dot rows,
        # so no [H, S] zhat is ever materialised.  The torch eps clamp is
        # inactive for this input regime (row norms are O(10)), so rsqrt
        # replaces sqrt+max+reciprocal.
        zwin = wpool.tile([H, WWIN], F32, tag="zwin")
        nrzwin = wpool.tile([1, WWIN], F32, tag="nrzwin")
        with tc.tile_pool(name="psumE", bufs=1, space="PSUM") as ppoolE:
            psum_zp = ppoolE.tile([H, S], F32, tag="psum_zp")
            nc.tensor.matmul(psum_zp[:], wzt_sb, zmT[:],
                             start=True, stop=True, skip_group_check=True)
            zps = wpool.tile([H, S], F32, tag="zps")
            gsum = wpool.tile([H, 1], F32, tag="gsum")
            nc.scalar.activation(zps[:], psum_zp[:], AF.Identity,
                                 bias=bz_sb, accum_out=gsum[:])

            # distance = sum((z_pool - gmean)^2)  (host divides by S)
            gmean_n = wpool.tile([H, 1], F32, tag="gmean_n")
            nc.scalar.mul(gmean_n[:], gsum[:], -1.0 / S)
            zc = wpool.tile([H, S], F32, tag="zc")
            nc.scalar.activation(zc[:], zps[:], AF.Identity, bias=gmean_n)
            dsq = wpool.tile([H, S], F32, tag="dsq")
            dssq = wpool.tile([H, 1], F32, tag="dssq")
            nc.scalar.activation(dsq[:], zc[:], AF.Square, accum_out=dssq[:])
            nc.tensor.matmul(psum_dist[:], dssq[:], ones_sb,
                             start=True, stop=True, skip_group_check=True)

            # per-column 1/||z_pool[s]||, kept as a row for late application
            sqz = wpool.tile([H, S], F32, tag="sqz")
            nc.scalar.activation(sqz[:], zps[:], AF.Square)
            psum_zn = ppoolE.tile([1, S], F32, tag="psum_zn")
            nc.tensor.matmul(psum_zn[:], ones_sb, sqz[:],
                             start=True, stop=True, skip_group_check=True)
            nrz = wpool.tile([1, S], F32, tag="nrz")
            nc.scalar.sqrt(nrz[:], psum_zn[:])

            # per-core window [t0+1, t0+WWIN] of raw zps columns and of the
            # rz row; t0 comes from the per-core uint32 input, so one
            # dynamic slice covers all 14 static shift slices below.
            treg = nc.scalar.alloc_register("t0_reg%d" % nc.next_id())
            nc.scalar.reg_load(treg, aps["toff"][0:1, 0:1])
            tval = nc.scalar.snap(treg, donate=True, min_val=0, max_val=TMAX)
            nc.scalar.copy(zwin[:], zps[:, 1:S][:, bass.ds(tval, WWIN)])
            nc.scalar.copy(nrzwin[:], nrz[0:1, 1:S][:, bass.ds(tval, WWIN)])

        # ------------- h pooling (vector folds + PE matvecs) ---------------
        hmT_sb = _emit_h_pool(nc, tc, aps, hbufs, wvec_sb, wpool)

        # ------------- NCE (t-sharded; local h only) -----------------------
        with tc.tile_pool(name="psumC", bufs=1, space="PSUM") as ppoolC:
            # context projection, feature-major [H, SH]
            psum_cph = ppoolC.tile([H, SH], F32, tag="psum_cph")
            nc.tensor.matmul(psum_cph[:], wct_sb[:], hmT_sb[:],
                             start=True, stop=False, skip_group_check=True)
            nc.tensor.matmul(psum_cph[:], bc_sb[:], ones_r_sb[0:1, 0:SH],
                             start=False, stop=True, skip_group_check=True)

            # column norms -> cphihat = c_phi * 1/||c_phi[t]||
            sqc = wpool.tile([H, SH], F32, tag="sqc")
            nc.scalar.activation(sqc[:], psum_cph[:], AF.Square)
            psum_cn = ppoolC.tile([1, SH], F32, tag="psum_cn")
            nc.tensor.matmul(psum_cn[:], ones_sb, sqc[:],
                             start=True, stop=True, skip_group_check=True)
            nrc = wpool.tile([1, SH], F32, tag="nrc")
            nc.scalar.sqrt(nrc[:], psum_cn[:])
            rc = wpool.tile([1, SH], F32, tag="rc")
            nc.vector.reciprocal(rc[:], nrc[:])
            psum_rcb = ppoolC.tile([H, SH], F32, tag="psum_rcb")
            nc.tensor.matmul(psum_rcb[:], ones_r_sb, rc[:],
                             start=True, stop=True, skip_group_check=True)
            rcb = wpool.tile([H, SH], F32, tag="rcb")
            nc.scalar.copy(rcb[:], psum_rcb[:])
            cph = wpool.tile([H, SH], F32, tag="cph")
            nc.vector.tensor_tensor(out=cph[:], in0=psum_cph[:], in1=rcb[:],
                                    op=OP.mult)

            # cosine sims: per shift one DVE multiply + one ones-matmul
            # column reduction; results land as [1, 45] blocks in PSUM,
            # then the rz window rows normalise them block-strided.
            psum_dp = ppoolC.tile([1, NPOS * SH], F32, tag="psum_dp")
            psum_dn = ppoolC.tile([1, NNEG * SH], F32, tag="psum_dn")
            for j, delta in enumerate(SHIFTS):
                prod = prodpool.tile([H, SH], F32, tag="prod")
                nc.vector.tensor_tensor(
                    out=prod[:], in0=zwin[:, delta - 1:delta - 1 + SH],
                    in1=cph[:], op=OP.mult)
                dst = (psum_dp[:, j * SH:(j + 1) * SH] if j < NPOS
                       else psum_dn[:, (j - NPOS) * SH:(j - NPOS + 1) * SH])
                nc.tensor.matmul(dst, ones_sb, prod[:],
                                 start=True, stop=True, skip_group_check=True)
            rzwin = wpool.tile([1, WWIN], F32, tag="rzwin")
            nc.vector.reciprocal(rzwin[:], nrzwin[:])
            rw = rzwin[:]
            dpos = wpool.tile([1, NPOS * SH], F32, tag="dpos")
            nc.vector.tensor_tensor(
                out=dpos[:], in0=psum_dp[:],
                in1=bass.AP(rw.tensor, rw.offset,
                            [[rw.ap[0][0], 1], [1, NPOS], [1, SH]]),
                op=OP.mult)
            dneg = wpool.tile([1, NNEG * SH], F32, tag="dneg")
            nc.vector.tensor_tensor(
                out=dneg[:], in0=psum_dn[:],
                in1=bass.AP(rw.tensor, rw.offset + SHIFTS[NPOS] - 1,
                            [[rw.ap[0][0], 1], [1, NNEG], [1, SH]]),
                op=OP.mult)

            # log-softmax over the 8 samples; positive at m=0
            expd = wpool.tile([1, NC14 * SH], F32, tag="expd")
            nc.scalar.activation(expd[0:1, 0:NPOS * SH], dpos[:], AF.Exp)
            nc.scalar.activation(expd[0:1, NPOS * SH:NC14 * SH], dneg[:],
                                 AF.Exp)
            den = wpool.tile([1, NPOS * SH], F32, tag="den")
            eb = expd[:]
            for ii in range(TIMESPAN):
                # negatives for step ii live in 7 consecutive 45-blocks
                neg_ap = bass.AP(
                    eb.tensor, eb.offset + (NPOS + ii) * SH,
                    [[eb.ap[0][0], 1], [1, SH], [SH, 7]])
                nc.vector.reduce_sum(den[0:1, ii * SH:(ii + 1) * SH],
                                     neg_ap, axis=AX.X)
            nc.vector.tensor_add(den[:], den[:], expd[0:1, 0:NPOS * SH])
            lse = wpool.tile([1, NPOS * SH], F32, tag="lse")
            nc.scalar.activation(lse[:], den[:], AF.Ln)
            ctr = wpool.tile([1, NPOS * SH], F32, tag="ctr")
            nc.vector.tensor_sub(ctr[:], dpos[:], lse[:])
            cb = ctr[:]
            ctrt = wpool.tile([1, SH], F32, tag="ctrt")
            sum_ap = bass.AP(cb.tensor, cb.offset,
                             [[cb.ap[0][0], 1], [1, SH], [SH, TIMESPAN]])
            nc.vector.reduce_sum(ctrt[:], sum_ap, axis=AX.X)
            masked = wpool.tile([1, SH], F32, tag="masked")
            nc.vector.tensor_tensor(out=masked[:], in0=ctrt[:], in1=mask_sb,
                                    op=OP.mult)
            nce1 = wpool.tile([1, 1], F32, tag="nce1")
            nc.vector.reduce_sum(nce1[:], masked[:], axis=AX.X)

        # ------------- raw partials out (host sums across cores) -----------
        out_sb = wpool.tile([1, 2], F32, tag="out_sb")
        nc.vector.tensor_copy(out=out_sb[0:1, 0:1], in_=nce1[:])
        nc.scalar.copy(out_sb[0:1, 1:2], psum_dist[:])
        nc.sync.dma_start(out[:], out_sb[:])


def _ap_of(t, off, dims):
    b = t[:]
    return bass.AP(b.tensor, b.offset + off, [[b.ap[0][0], b.ap[0][1]]] + dims)


def _emit_h_pool(nc, tc, aps, hbufs, wvec_sb, wpool):
    """Segmented unit-stride folds (8 nodes/partition -> 1) on the vector
    engine, then one [128]x[128,1] matvec per timestep on PE.  Returns
    hmT_sb [H, SH] (the node-mean, transposed)."""
    with tc.tile_pool(name="hacc", bufs=1) as hapool, \
         tc.tile_pool(name="psumH", bufs=1, space="PSUM") as ppoolH:
        hacc = hapool.tile([128, SH * H], F32, tag="hacc")
        psum_hmT = ppoolH.tile([H, SH], F32, tag="psum_hmT")
        for g, hbuf in enumerate(hbufs):
            if _OPTS["dma_only"]:
                continue
            # fold u: 8 -> 4 (into the tail region), 4 -> 2 (back to base),
            # 2 -> 1 (straight into hacc)
            nc.vector.tensor_add(
                _ap_of(hbuf, HCHUNK, [[4 * H, HS], [1, 4 * H]]),
                _ap_of(hbuf, 0, [[NB * H, HS], [1, 4 * H]]),
                _ap_of(hbuf, 4 * H, [[NB * H, HS], [1, 4 * H]]))
            nc.vector.tensor_add(
                _ap_of(hbuf, 0, [[2 * H, HS], [1, 2 * H]]),
                _ap_of(hbuf, HCHUNK, [[4 * H, HS], [1, 2 * H]]),
                _ap_of(hbuf, HCHUNK + 2 * H, [[4 * H, HS], [1, 2 * H]]))
            nc.vector.tensor_add(
                _ap_of(hacc, g * HS * H, [[H, HS], [1, H]]),
                _ap_of(hbuf, 0, [[2 * H, HS], [1, H]]),
                _ap_of(hbuf, H, [[2 * H, HS], [1, H]]))
            for k in range(HS):
                s = g * HS + k
                nc.tensor.matmul(
                    psum_hmT[:, s:s + 1],
                    hacc[:, s * H:(s + 1) * H], wvec_sb,
                    start=True, stop=True, skip_group_check=True)
        if _OPTS["dma_only"] or wpool is None:
            return None
        hmT_sb = wpool.tile([H, SH], F32, tag="hmT_sb")
        nc.scalar.copy(hmT_sb[:], psum_hmT[:])
        return hmT_sb


def _build(reps=1):
    nc = bacc.Bacc("TRN2", debug=False, enable_asserts=False,
                   target_bir_lowering=False, num_devices=NCORES)
    aps = {}

    def din(name, shape, dt=F32):
        aps[name] = nc.dram_tensor(name, shape, dt, kind="ExternalInput").ap()

    din("ahd", [SH, 128, NB * H])
    din("azd", [128, ZCOLS])
    din("consts", [128, CW])
    din("toff", [1, 1], U32)
    aps["out"] = nc.dram_tensor("out", [1, 2], F32,
                                kind="ExternalOutput").ap()

    with tile.TileContext(nc) as tc:
        _emit(nc, tc, aps, reps=reps)
    nc.compile()
    return nc


_CACHE = {}


def _core_t0(c):
    """Global first t_sample of core c's shard (core 7 clipped to END-45)."""
    return min(START + SH * c, END - SH)


def _pack_consts(Wh, bh, Wz, bz, Wphi, bphi, core):
    c = np.zeros((128, CW), dtype=np.float32)
    c[0:Z, _C_WZT:_C_WZT + H] = Wz.T
    c[:, _C_WH:_C_WH + H] = Wh
    c[:, _C_WPHIT:_C_WPHIT + H] = Wphi.T
    c[:, _C_BZ] = bz
    c[:, _C_BH] = bh
    c[:, _C_ONES] = 1.0
    c[:, _C_WVEC] = 1.0 / N
    for q in range(128):
        c[q, _C_PICK + q // 2] = 1.0 / N
    c[0, _C_ONES_R:_C_ONES_R + H] = 1.0
    c[0, _C_BPHI:_C_BPHI + H] = bphi
    # rows whose global index is owned by a lower core are masked off
    # (only core 7's clipped shard overlaps core 6's)
    t0 = _core_t0(core)
    m = (np.arange(t0, t0 + SH) >= START + SH * core).astype(np.float32)
    c[0, _C_MASK:_C_MASK + SH] = m
    return c


def make_in_maps(all_h, all_z, Wh, bh, Wz, bz, Wphi, bphi):
    in_maps = []
    for c in range(NCORES):
        t0 = _core_t0(c)
        in_maps.append({
            "consts": _pack_consts(Wh, bh, Wz, bz, Wphi, bphi, c),
            "toff": np.array([[t0]], dtype=np.uint32),
            "ahd": np.ascontiguousarray(
                all_h[t0:t0 + SH]).reshape(SH, 128, NB * H),
            "azd": np.ascontiguousarray(
                all_z[c * SZ:(c + 1) * SZ]).reshape(128, ZCOLS),
        })
    return in_maps


def _get_runner():
    """Build the Bass program and one jitted shard_map executable, once.

    Re-lowering a fresh executable per call reloads the collective NEFF and
    leaves NRT unrecoverable on the second call, so the executable is cached
    and every kernel() invocation reuses it with freshly uploaded inputs.
    """
    if "runner" in _CACHE:
        return _CACHE["runner"]

    import jax
    from concourse import bass2jax
    from concourse.bass2jax import _bass_exec_p, partition_id_tensor
    from jax.sharding import Mesh, PartitionSpec, NamedSharding
    from jax.experimental.shard_map import shard_map

    nc = _build()
    bass2jax.install_neuronx_cc_hook()
    partition_name = (nc.partition_id_tensor.name
                      if nc.partition_id_tensor else None)

    in_names, out_names, out_avals, zero_outs = [], [], [], []
    for alloc in nc.m.functions[0].allocations:
        if not isinstance(alloc, mybir.MemoryLocationSet):
            continue
        name = alloc.memorylocations[0].name
        if alloc.kind == "ExternalInput":
            if name != partition_name:
                in_names.append(name)
        elif alloc.kind == "ExternalOutput":
            shape = tuple(alloc.tensor_shape)
            dtype = mybir.dt.np(alloc.dtype)
            out_names.append(name)
            out_avals.append(jax.core.ShapedArray(shape, dtype))
            zero_outs.append(np.zeros(shape, dtype))
    n_params = len(in_names)
    all_in_names = list(in_names) + out_names
    if partition_name is not None:
        all_in_names.append(partition_name)

    def _body(*args):
        operands = list(args)
        if partition_name is not None:
            operands.append(partition_id_tensor())
        outs = _bass_exec_p.bind(
            *operands,
            out_avals=tuple(out_avals),
            in_names=tuple(all_in_names),
            out_names=tuple(out_names),
            lowering_input_output_aliases=(),
            sim_require_finite=True,
            sim_require_nnan=True,
            nc=nc,
        )
        return tuple(outs)

    devices = jax.devices()[:NCORES]
    mesh = Mesh(np.asarray(devices), ("core",))
    n_outs = len(out_avals)
    in_specs = (PartitionSpec("core"),) * (n_params + n_outs)
    out_specs = (PartitionSpec("core"),) * n_outs
    sharded = jax.jit(shard_map(_body, mesh=mesh, in_specs=in_specs,
                                out_specs=out_specs, check_rep=False),
                      keep_unused=True)
    sh = NamedSharding(mesh, PartitionSpec("core"))
    dev_zeros = [
        jax.device_put(
            np.zeros((NCORES * z.shape[0], *z.shape[1:]), z.dtype), sh)
        for z in zero_outs
    ]

    def run(in_maps):
        dev_in = [
            jax.device_put(
                np.concatenate([np.asarray(in_maps[c][n])
                                for c in range(NCORES)], axis=0), sh)
            for n in in_names
        ]
        outs = sharded(*dev_in, *dev_zeros)
        return [
            {name: np.asarray(outs[i]).reshape(NCORES, *out_avals[i].shape)[c]
             for i, name in enumerate(out_names)}
            for c in range(NCORES)
        ]

    _CACHE["runner"] = run
    return run


def kernel(all_h, all_z, Wh, bh, Wz, bz, Wphi, bphi):
    all_h = np.ascontiguousarray(np.asarray(all_h, dtype=np.float32))
    all_z = np.ascontiguousarray(np.asarray(all_z, dtype=np.float32))
    args = [np.asarray(x, dtype=np.float32)
            for x in (Wh, bh, Wz, bz, Wphi, bphi)]

    # The axon NTFF trace hook (antenv.axon_hooks) is absent in this image;
    # make sure an inherited BASS_TRACE can't route us onto that path.
    os.environ["BASS_NEVER_TRACE"] = "1"

    run = _get_runner()
    in_maps = make_in_maps(all_h, all_z, *args)
    results = run(in_maps)
    _CACHE["last_results"] = results

    # unshard: sum per-core NCE partials, average the replicated distance
    nce_sum = float(sum(results[c]["out"][0, 0] for c in range(NCORES)))
    dist_sum = float(sum(results[c]["out"][0, 1] for c in range(NCORES)))
    nce_loss = np.float32(nce_sum * (-1.0 / (CNT * TIMESPAN)))
    distance = np.float32(dist_sum / (NCORES * S))
    return (nce_loss, distance)
